# revision 26
# baseline (speedup 1.0000x reference)
"""AttentionalJoin kernel for 8 Trainium2 NeuronCores.

Math: the reference builds full (M x M) self-attention over M = N+1 tokens
(CLS prepended) but returns only the CLS row of the projected output.  Only
the CLS query survives, so attention collapses to a softmax-weighted token
pooling:

    q       = Wq @ cls                       (per head h: q_h)
    score_t = scale * q_h . (Wk x_t)_h  =  x_t . R[:, h],   R = scale*Wk_h^T q_h
    p       = softmax over the M tokens
    pooled_h = sum_t p_t x_t                 (linearity: project AFTER pooling)
    out     = proj( concat_h Wv_h pooled_h ) + proj_b

Device design (v2): x streams from HBM ONCE in fp8e3 (e3m4) but in BOTH
layouts (natural [t, c] and transposed [c, t]) so that neither matmul needs
an on-chip transpose or a PSUM drain of x:

  scores : per t-block, 4 accumulating matmuls with the x^T block as
           fp8 stationary weights (fast-weight-load, 27 ns) and the 8-col
           R chunk as fp16 moving operand -> PSUM [t, 8]
  exp    : one ACT op per 4 t-blocks, exp(s - 3) -> E fp16 in SBUF
           (the -3 bias keeps e^s in fp16 range; it cancels in p = e/Z)
  pool   : per t-block, 4 matmuls with the natural x block as fp8
           stationary and E [t, 8] fp16 moving -> PSUM [c, 8], one
           accumulation group per (batch, c-chunk)
  Z      : per 4-block group, one matmul with E as stationary [t, 32]
           and a ones column moving -> PSUM [32, s]

PE cost ~ 264 small matmuls dominated by fp8 FWL weight loads (~27 ns each);
DMA cost ~ 4.3 MB -> the kernel is DMA-bound near the fp8 memory roofline.
e3m4 keeps end-to-end rel err ~1e-2 (e4m3 would be ~2e-2, at the gate).
The tiny tail (CLS term, 1/Z, head-mix, proj, bias) runs on host.

Sharding: data-parallel over the batch dim, 2 batches per core.
"""

import numpy as np

H = 8
C = 512
HD = C // H
B = 16
N = 2048
NCORES = 8
BPC = B // NCORES          # batches per core
TOK = BPC * N              # tokens per core (4096)
NTB = TOK // 128           # 128-token blocks per core (32)
NSG = NTB // 4             # 4-block score/pool groups (8; 4 per batch)
SGB = NSG // BPC           # groups per batch (4)
EXP_BIAS = -3.0
MAX_DRAIN_WAITS = 1        # this walrus rejects instructions w/ >1 sem wait

_cached = {}


def _patch_drain():
    """The container's walrus codegen rejects instructions carrying more
    than one sem wait ("Too many sync wait commands").  Split extra waits
    onto dedicated same-engine NOPs, which preserves semantics (engine
    queues are in-order)."""
    import concourse.tile as tile_mod
    from concourse import mybir
    from bass_rust import ScopedClock

    if getattr(tile_mod.TileContext, "_drain_patched", False):
        return

    orig_lower = tile_mod.TileContext._lower_ordered_insts

    def _lower_ordered_insts(self, ordered):
        nc = self.nc
        for bbname, insts in ordered.items():
            out = []
            for inst in insts:
                si = inst.sync_info
                if si is not None and si.on_wait and len(si.on_wait) > MAX_DRAIN_WAITS:
                    waits = list(si.on_wait)
                    extra, keep = waits[:-MAX_DRAIN_WAITS], waits[-MAX_DRAIN_WAITS:]
                    for w in extra:
                        nop = mybir.InstNoOp(
                            name=f"waitsplit-{nc.next_id()}",
                            engine=inst.engine,
                            ins=[],
                            outs=[],
                            bass_nofuse=True,
                            sync_info=mybir.SyncInfo(on_wait=[w], on_update=[]),
                            debug=inst.debug,
                        )
                        out.append(nop)
                    inst.sync_info = mybir.SyncInfo(
                        on_wait=keep, on_update=list(si.on_update)
                    )
                out.append(inst)
            ordered[bbname] = out
        return orig_lower(self, ordered)

    tile_mod.TileContext._lower_ordered_insts = _lower_ordered_insts

    def _drain_and_barrier(self, tick_clock, wait_clock):
        nc = self.nc
        probe = mybir.InstNoOp(
            name=f"drain-wait-probe-{nc.next_id()}",
            engine=mybir.EngineType.SP,
            ins=[],
            outs=[],
        )
        wait_clock.add_sem_waits(probe, ScopedClock({None: tick_clock.global_clock}))
        waits = list(probe.sync_info.on_wait) if probe.sync_info else []
        for i in range(0, len(waits), MAX_DRAIN_WAITS):
            chunk = waits[i : i + MAX_DRAIN_WAITS]
            nop = nc.sync.nop(nofuse=True, hint="drain_wait")
            nop.ins.sync_info = mybir.SyncInfo(on_wait=chunk, on_update=[])
        nc.sync.drain()

        nc.all_engine_barrier()
        popped = nc._tile_sem_poison_stack.pop()
        assert popped is self._sem_poison
        nc.clear_and_free_semaphores(list(self.sems.allocated().values()))
        nc.all_engine_barrier()

    tile_mod.TileContext._drain_and_barrier = _drain_and_barrier
    tile_mod.TileContext._drain_patched = True


def _build_module():
    import concourse.bass as bass
    import concourse.tile as tile
    from concourse import mybir
    from concourse.masks import make_identity

    _patch_drain()
    f8 = mybir.dt.float8e3
    f16 = mybir.dt.float16
    f32 = mybir.dt.float32
    EXP = mybir.ActivationFunctionType.Exp

    nc = bass.Bass()
    # x^T, partition-major: [128 c][cq][4096 t]  (sliced into 4 t-super DMAs)
    xt_in = nc.dram_tensor("xt", [128, 4, TOK], f8, kind="ExternalInput")
    # x natural, partition-major: [128 t][s8][4 tb][512 c]  (3 s-range DMAs)
    xn_in = nc.dram_tensor("xn", [128, NSG, 4, C], f8, kind="ExternalInput")
    # R: [cq][128 c][8 h] fp16
    r_in = nc.dram_tensor("r", [4, 128, H], f16, kind="ExternalInput")
    # pooled sums batch 0: [128 c-within-chunk][cq][h]
    p0_out = nc.dram_tensor("pool0", [128, 4, H], f32, kind="ExternalOutput")
    # batch 1 + Z partials ([32, NSG] tucked into slot 4): [128][5][h]
    p1_out = nc.dram_tensor("pool1", [128, 5, H], f32, kind="ExternalOutput")

    with tile.TileContext(nc) as tc:
        with (
            tc.tile_pool(name="consts", bufs=1) as consts,
            tc.tile_pool(name="xtp", bufs=1) as xtp,
            tc.tile_pool(name="xnp", bufs=1) as xnp,
            tc.tile_pool(name="ep", bufs=1) as ep,
            tc.tile_pool(name="op", bufs=1) as op,
            tc.tile_pool(name="psc", bufs=2, space="PSUM") as psc_pool,
            tc.tile_pool(name="pht", bufs=1, space="PSUM") as pht_pool,
            tc.tile_pool(name="ppl", bufs=1, space="PSUM") as ppl_pool,
            tc.tile_pool(name="pzp", bufs=1, space="PSUM") as pzp_pool,
        ):
            xt_src = xt_in.rearrange("p q f -> p q f")
            xn_src = xn_in.rearrange("p s a f -> p s a f")

            # R rides on the ACT ring (ahead of the exps); the 7 x-input
            # doorbells all go on SP: the qSPDynamicHW ring accepts ~7
            # entries before the issuing sequencer stalls, and doorbells on
            # ACT would head-of-line-block the exp stream
            r_sb = consts.tile([128, 4, H], f16)
            nc.scalar.dma_start(out=r_sb, in_=r_in.rearrange("q p h -> p q h"))

            # x^T: [c-part, cq, t] tile; one 512KB DMA per t-super so
            # scores unlock progressively while the stream flows
            xt_sb = xtp.tile([128, 4, TOK], f8, tag="xt", name="xt")
            for s4 in range(4):
                nc.sync.dma_start(
                    out=xt_sb[:, :, s4 * 1024 : (s4 + 1) * 1024],
                    in_=xt_src[:, :, s4 * 1024 : (s4 + 1) * 1024],
                )
            # x natural tiles (pooling stationaries): 3 DMAs (3+3+1 tiles)
            # on SP; the SP HWDGE ring stalls beyond ~7 outstanding entries.
            # Tile 7 rides the ACT ring mid-stream (doorbell emitted below,
            # between the exps) so its data+sem land well before the end —
            # the post-stream critical path is then just pool group 6.
            xn_sb = xnp.tile([128, NSG, 4, C], f8, tag="xn", name="xn")
            for lo, hi in ((0, 3), (3, 6), (6, 7)):
                nc.sync.dma_start(out=xn_sb[:, lo:hi], in_=xn_src[:, lo:hi])

            ident = consts.tile([128, 128], f16)
            make_identity(nc, ident)
            bias_t = consts.tile([128, 1], f32)
            nc.vector.memset(bias_t, EXP_BIAS)
            ones = consts.tile([128, 1], f16)
            nc.vector.memset(ones, 1.0)

            # ACT warm-up: triggers the 1.3µs exp LUT table load while the
            # x stream is still in flight
            warm = consts.tile([128, 1], f16)
            nc.scalar.activation(out=warm, in_=bias_t, func=EXP, bias=bias_t, scale=1.0)

            # HAM heaters: keep the PE streaming while the first x pieces
            # land so the clock gate is at 8/8 when real work starts
            ht = pht_pool.tile([128, 128], f32, tag="heat", name="heat")
            NHEAT = 14
            for k in range(NHEAT):
                nc.tensor.matmul(ht, ident, ident, start=(k == 0), stop=(k == NHEAT - 1))

            # pooled accumulator [c-part, b, q, h]: one PSUM bank
            pp = ppl_pool.tile([128, BPC, 4, H], f32, tag="pp", name="pp")
            # Z partials [32, group]: one PSUM bank
            pz = pzp_pool.tile([32, NSG], f32, tag="pz", name="pz")

            es = {}

            def stage_scores(s):
                """scores for group s (4 t-blocks): 16 matmuls, x^T blocks
                stationary (fp8 FWL), R chunks moving; then one ACT exp."""
                ps = psc_pool.tile([128, 4, H], f32, tag="ps", name=f"ps{s}")
                for a in range(4):
                    tb = s * 4 + a
                    for q in range(4):
                        nc.tensor.matmul(
                            ps[:, a, :],
                            xt_sb[:, q, tb * 128 : (tb + 1) * 128],
                            r_sb[:, q, :],
                            start=(q == 0),
                            stop=(q == 3),
                        )
                et = ep.tile([128, 4, H], f16, tag=f"e{s}", name=f"e{s}")
                nc.scalar.activation(out=et, in_=ps, func=EXP, bias=bias_t, scale=1.0)
                es[s] = et

            def stage_pool(s, first, last):
                """pooled += x_block^T @ E_block for the 4 t-blocks of group
                s (natural x stationary, E moving).  The group Z column goes
                FIRST so pz's final stop lands before the last pool matmuls
                and its drain copy overlaps them."""
                b = s // SGB
                et = es[s]
                nc.tensor.matmul(
                    pz[:, s : s + 1], et, ones, start=True, stop=True
                )
                for a in range(4):
                    for q in range(4):
                        nc.tensor.matmul(
                            pp[:, b, q, :],
                            xn_sb[:, s, a, q * 128 : (q + 1) * 128],
                            et[:, a, :],
                            start=(first and a == 0 and q == 0),
                            stop=(last and a == 3 and q == 3),
                        )

            # emission order matches data arrival: xt pieces unlock scores,
            # then the xn ranges unlock pool groups.  Group 7's tile comes
            # early via the ACT ring, so it is emitted BEFORE group 6 and
            # group 6 carries batch 1's accumulation stop.
            for s in range(NSG):
                stage_scores(s)
                if s == 5:
                    # ACT-ring doorbell for xn tile 7, squeezed between the
                    # exps: the ACT HWDGE ring only holds R, so this fires
                    # immediately and the data interleaves into the stream
                    nc.scalar.dma_start(out=xn_sb[:, 7:8], in_=xn_src[:, 7:8])
            so1 = op.tile([128, 5, H], f32, tag="so1", name="so1")
            for s, first, last in (
                (0, True, False), (1, False, False), (2, False, False),
                (3, False, True),
                (4, True, False), (5, False, False), (7, False, False),
                (6, False, True),
            ):
                stage_pool(s, first, last)
                if s == SGB - 1:
                    so0 = op.tile([128, 4, H], f32, tag="so0", name="so0")
                    nc.vector.tensor_copy(so0, pp[:, 0])
                    nc.scalar.dma_start(out=p0_out.rearrange("p a h -> p a h"), in_=so0)
            nc.vector.tensor_copy(so1[0:32, 4, :], pz)
            nc.vector.tensor_copy(so1[:, 0:4, :], pp[:, 1])
            nc.scalar.dma_start(out=p1_out.rearrange("p a h -> p a h"), in_=so1)

    return nc


def _get_module():
    if "nc" not in _cached:
        _cached["nc"] = _build_module()
    return _cached["nc"]


def _host_prep(cls, qkv_w):
    scale = HD ** -0.5
    c = cls.reshape(C).astype(np.float64)
    Wq = qkv_w[:C].astype(np.float64)
    Wk = qkv_w[C : 2 * C].astype(np.float64)
    q = Wq @ c
    qh = q.reshape(H, HD)
    Wkh = Wk.reshape(H, HD, C)
    R = (scale * np.einsum("hdc,hd->ch", Wkh, qh)).astype(np.float16)
    k0 = Wk @ c
    score0 = scale * np.einsum("hd,hd->h", qh, k0.reshape(H, HD))
    e0 = np.exp(score0 + EXP_BIAS)
    return R, e0


def prepare_in_maps(x, cls, qkv_w):
    """Quantize x to e3m4 and build the per-core input maps (both layouts)."""
    import ml_dtypes

    R, e0 = _host_prep(cls, qkv_w)
    xq = np.ascontiguousarray(x.reshape(B * N, C)).astype(ml_dtypes.float8_e3m4)
    r_dev = np.ascontiguousarray(R.reshape(4, 128, H))
    in_maps = []
    for i in range(NCORES):
        xc = xq[i * TOK : (i + 1) * TOK]                       # [4096, 512]
        # partition-major layouts: xn[p][s][a][f], xt[p][q][t]
        xn = np.ascontiguousarray(xc.reshape(NSG, 4, 128, C).transpose(2, 0, 1, 3))
        xt = np.ascontiguousarray(xc.T.reshape(4, 128, TOK).swapaxes(0, 1))
        in_maps.append({"xt": xt, "xn": xn, "r": r_dev})
    return in_maps, e0


def kernel(x, cls, qkv_w, proj_w, proj_b):
    from concourse.bass_utils import run_bass_kernel_spmd

    x = np.asarray(x, dtype=np.float32)
    cls = np.asarray(cls, dtype=np.float32)
    qkv_w = np.asarray(qkv_w, dtype=np.float32)
    proj_w = np.asarray(proj_w, dtype=np.float32)
    proj_b = np.asarray(proj_b, dtype=np.float32)

    in_maps, e0 = prepare_in_maps(x, cls, qkv_w)
    Wv = qkv_w[2 * C :]

    nc = _get_module()
    res = run_bass_kernel_spmd(nc, in_maps, list(range(NCORES)))
    _cached["last_results"] = res

    s_parts = []
    z_parts = []
    for i in range(NCORES):
        p0 = res.results[i]["pool0"]         # [128, 4, H]
        p1z = res.results[i]["pool1"]        # [128, 5, H]
        p_dev = np.stack([p0, p1z[:, :4, :]])  # [BPC, 128, 4, H]
        z_dev = p1z[:32, 4, :]               # [32, NSG] = [(a, h), group]
        # pooled[b, h, c] with c = q*128 + p
        s_parts.append(np.transpose(p_dev, (0, 3, 2, 1)).reshape(BPC, H, C))
        zg = z_dev.reshape(4, H, BPC, SGB)   # [(a), h, b, g]
        z_parts.append(zg.sum(axis=(0, 3)).T)  # [b, h]
    s_dev = np.concatenate(s_parts, axis=0)  # [B, H, C]
    z_dev = np.concatenate(z_parts, axis=0)  # [B, H]

    # add the CLS token's own contribution, normalize, head-mix + proj
    cf = cls.reshape(C).astype(np.float64)
    s_full = s_dev.astype(np.float64) + (e0[:, None] * cf[None, :])[None]
    z_full = z_dev.astype(np.float64) + e0[None]
    v = s_full / z_full[:, :, None]
    o = np.einsum("hdc,bhc->bhd", Wv.astype(np.float64).reshape(H, HD, C), v)
    y = o.reshape(B, C) @ proj_w.T.astype(np.float64) + proj_b.astype(np.float64)
    return y.astype(np.float32)
